# revision 22
# baseline (speedup 1.0000x reference)
"""Adaptive graph pooling (gnn_message_passing) on 8 TRN2 NeuronCores.

Sharding: nodes 256-per-core.  Host work is limited to sharding / index prep
(CSR/CSC bucketing of the edge list) and output assembly.

v2 pipeline (collectives start as early as their inputs allow; S-dependent
matmuls are decomposed so the big AllGathers ship cm-independent tensors):

  S = FS .* cm[col] + diag(diagv)          (FS = multiplicity-weighted fitness)
  AS[:,own]   = cm[own]*(A @ FS[:,own]) + A[:,own]*diagv[own]
  Emat[:,own] = cm[row] * (FS^T @ AS) + diagv[row] * AS

  collective order: AG(A^T) | AllReduce(colsums) | A2A(FS,A cols) | AG(cm)
                    | AG(FS) | AG(diagv)
  mm1 = A @ FS[:,own] runs while AG(FS)/AG(cm) are on the wire.
"""
import sys
if '/opt/trn_rl_repo' not in sys.path:
    sys.path.insert(0, '/opt/trn_rl_repo')

import numpy as np
import ml_dtypes

import concourse.bass as bass
import concourse.tile as tile
from concourse.tile import add_dep_helper
from concourse import bacc, mybir, library_config
from concourse.bass_utils import run_bass_kernel_spmd

F32 = mybir.dt.float32
BF16 = mybir.dt.bfloat16
I16 = mybir.dt.int16
U16 = mybir.dt.uint16
AX = mybir.AxisListType
OP = mybir.AluOpType
ACT = mybir.ActivationFunctionType

N = 2048
D = 512
NCORES = 8
ROWS = N // NCORES
P = 128
RT = ROWS // P
KT = N // P
DT = D // P

_cache = {}


def _rsqrt(nc, ss, col):
    """ss[:, col] = sumsq -> returns AP of 1/max(sqrt(ss),1e-12)."""
    c = col
    nc.scalar.activation(ss[:, c + 1:c + 2], ss[:, c:c + 1], ACT.Sqrt)
    nc.vector.tensor_scalar_max(ss[:, c + 2:c + 3], ss[:, c + 1:c + 2], 1e-12)
    nc.vector.reciprocal(ss[:, c + 3:c + 4], ss[:, c + 2:c + 3])
    return ss[:, c + 3:c + 4]


def _build(K_csr, K_csc):
    nc = bacc.Bacc("TRN2", target_bir_lowering=False, debug=False,
                   enable_asserts=False, num_devices=NCORES)

    # ---- I/O ----
    emb_in = nc.dram_tensor("emb", [N, D], F32, kind="ExternalInput").ap()
    emb_own_in = nc.dram_tensor("emb_own", [ROWS, D], F32, kind="ExternalInput").ap()
    csr_h0 = nc.dram_tensor("csr_h0", [ROWS, K_csr], I16, kind="ExternalInput").ap()
    csr_h1 = nc.dram_tensor("csr_h1", [ROWS, K_csr], I16, kind="ExternalInput").ap()
    csr_mult = nc.dram_tensor("csr_mult", [ROWS, K_csr], BF16, kind="ExternalInput").ap()
    uni_in = nc.dram_tensor("uni_idx", [ROWS, K_csr], U16, kind="ExternalInput").ap()
    csc_h0 = nc.dram_tensor("csc_h0", [ROWS, K_csc], I16, kind="ExternalInput").ap()
    csc_h1 = nc.dram_tensor("csc_h1", [ROWS, K_csc], I16, kind="ExternalInput").ap()
    csc_mult = nc.dram_tensor("csc_mult", [ROWS, K_csc], BF16, kind="ExternalInput").ap()
    dcj_s = nc.dram_tensor("diag_cj_s", [P, RT], F32, kind="ExternalInput").ap()
    dcj_e = nc.dram_tensor("diag_cj_e", [P, KT], F32, kind="ExternalInput").ap()

    s_out = nc.dram_tensor("s_out", [ROWS, N], F32, kind="ExternalOutput").ap()
    emat_out = nc.dram_tensor("emat_out", [N, ROWS], F32, kind="ExternalOutput").ap()
    pooled_out = nc.dram_tensor("pooled_out", [ROWS, D], F32, kind="ExternalOutput").ap()
    fit_out = nc.dram_tensor("fit_out", [ROWS, K_csr], F32, kind="ExternalOutput").ap()
    cmask_out = nc.dram_tensor("cmask_out", [P, RT], F32, kind="ExternalOutput").ap()

    # ---- collective bounces + local scratch (internal DRAM) ----
    agat_in = nc.dram_tensor("agat_in", [ROWS, N], BF16).ap()
    agat_out = nc.dram_tensor("agat_out", [N, N], BF16, addr_space="Shared").ap()
    ar_in = nc.dram_tensor("ar_in", [2, N], F32).ap()
    ar_out = nc.dram_tensor("ar_out", [2, N], F32, addr_space="Shared").ap()
    a2a_in = nc.dram_tensor("a2a_in", [NCORES, ROWS, ROWS], BF16).ap()
    a2a_out = nc.dram_tensor("a2a_out", [NCORES, ROWS, ROWS], BF16).ap()
    agcm_in = nc.dram_tensor("agcm_in", [ROWS], F32).ap()
    agcm_out = nc.dram_tensor("agcm_out", [N], F32, addr_space="Shared").ap()
    agfs_in = nc.dram_tensor("agfs_in", [ROWS, N], BF16).ap()
    agfs_out = nc.dram_tensor("agfs_out", [N, N], BF16, addr_space="Shared").ap()
    agdv_in = nc.dram_tensor("agdv_in", [ROWS], F32).ap()
    agdv_out = nc.dram_tensor("agdv_out", [N], F32, addr_space="Shared").ap()
    scr_cm = nc.dram_tensor("scr_cm", [2, ROWS], F32).ap()     # cm_own | diagv_own
    warm_in = nc.dram_tensor("warm_in", [64], F32).ap()
    warm_out = nc.dram_tensor("warm_out", [512], F32, addr_space="Shared").ap()
    scr_sc = nc.dram_tensor("scr_sc", [N], F32).ap()           # scores roundtrip

    # ---- constants ----
    jidx_np = np.broadcast_to(np.arange(N, dtype=np.float32), (P, N)).copy()
    i128_np = np.eye(P, dtype=np.float32)
    m16_np = np.zeros((P, 16), np.float32)
    m16_np[np.arange(P), np.arange(P) % 16] = 1.0
    jidx_c = nc.inline_tensor(jidx_np, "jidx_c").ap()
    i128_c = nc.inline_tensor(i128_np, "i128_c").ap()
    i128b_c = nc.inline_tensor(i128_np.astype(ml_dtypes.bfloat16), "i128b_c").ap()
    m16_c = nc.inline_tensor(m16_np, "m16_c").ap()

    rg = [list(range(NCORES))]

    with tile.TileContext(nc) as tc:
        with tc.tile_pool(name="const", bufs=1) as cpool, \
             tc.tile_pool(name="persist", bufs=1) as pp, \
             tc.tile_pool(name="small", bufs=1) as sp:

            nc.gpsimd.load_library(library_config.local_scatter)
            wz = sp.tile([1, 64], F32, tag="wz")
            nc.vector.memset(wz[:], 0.0)
            nc.sync.dma_start(warm_in[:], wz[:])
            nc.gpsimd.collective_compute("AllGather", OP.bypass, replica_groups=rg,
                                         ins=[warm_in[:].opt()], outs=[warm_out[:].opt()])

            jidx = cpool.tile([P, N], F32)
            i128 = cpool.tile([P, P], F32)
            i128b = cpool.tile([P, P], BF16)
            m16 = cpool.tile([P, 16], F32)
            ones_f = cpool.tile([P, 1], F32)
            ones_b = cpool.tile([P, 1], BF16)
            nc.sync.dma_start(jidx[:], jidx_c[:])
            nc.sync.dma_start(i128[:], i128_c[:])
            nc.sync.dma_start(i128b[:], i128b_c[:])
            nc.sync.dma_start(m16[:], m16_c[:])
            nc.vector.memset(ones_f[:], 1.0)
            nc.vector.memset(ones_b[:], 1.0)

            embb = [pp.tile([P, D], BF16, tag=f"embb{t}", name=f"embb{t}") for t in range(KT)]
            stat = [sp.tile([P, 16], F32, tag=f"stat{rt}", name=f"stat{rt}") for rt in range(RT)]
            zs = [sp.tile([P, 8], F32, tag=f"zs{rt}", name=f"zs{rt}") for rt in range(RT)]
            cmk_all = sp.tile([P, KT], F32, tag="cmk_all")

            bc1_cm = tc.tile_pool(name="bc1", bufs=1)
            bc1 = bc1_cm.__enter__()
            fraw = [bc1.tile([P, N], F32, tag=f"fraw{rt}", name=f"fraw{rt}") for rt in range(RT)]
            pbf = [bc1.tile([P, N], BF16, tag=f"pbf{rt}", name=f"pbf{rt}") for rt in range(RT)]
            atb = [bc1.tile([P, N], BF16, tag=f"atb{rt}", name=f"atb{rt}") for rt in range(RT)]

            # ---- edge scatters (gpsimd; independent, start immediately) ----
            for rt in range(RT):
                r0 = rt * P
                ih0 = sp.tile([P, K_csr], I16, tag=f"ih0{rt}", name=f"ih0{rt}")
                ih1 = sp.tile([P, K_csr], I16, tag=f"ih1{rt}", name=f"ih1{rt}")
                imu = sp.tile([P, K_csr], BF16, tag=f"imu{rt}", name=f"imu{rt}")
                nc.sync.dma_start(ih0[:], csr_h0[r0:r0 + P, :])
                nc.sync.dma_start(ih1[:], csr_h1[r0:r0 + P, :])
                nc.sync.dma_start(imu[:], csr_mult[r0:r0 + P, :])
                nc.gpsimd.local_scatter(out_ap=pbf[rt][:, 0:N // 2], data_ap=imu[:],
                                        idxs_ap=ih0[:], channels=P,
                                        num_elems=N // 2, num_idxs=K_csr)
                nc.gpsimd.local_scatter(out_ap=pbf[rt][:, N // 2:N], data_ap=imu[:],
                                        idxs_ap=ih1[:], channels=P,
                                        num_elems=N // 2, num_idxs=K_csr)
                ch0 = sp.tile([P, K_csc], I16, tag=f"ch0{rt}", name=f"ch0{rt}")
                ch1 = sp.tile([P, K_csc], I16, tag=f"ch1{rt}", name=f"ch1{rt}")
                cmu = sp.tile([P, K_csc], BF16, tag=f"cmu{rt}", name=f"cmu{rt}")
                nc.sync.dma_start(ch0[:], csc_h0[r0:r0 + P, :])
                nc.sync.dma_start(ch1[:], csc_h1[r0:r0 + P, :])
                nc.sync.dma_start(cmu[:], csc_mult[r0:r0 + P, :])
                nc.gpsimd.local_scatter(out_ap=atb[rt][:, 0:N // 2], data_ap=cmu[:],
                                        idxs_ap=ch0[:], channels=P,
                                        num_elems=N // 2, num_idxs=K_csc)
                nc.gpsimd.local_scatter(out_ap=atb[rt][:, N // 2:N], data_ap=cmu[:],
                                        idxs_ap=ch1[:], channels=P,
                                        num_elems=N // 2, num_idxs=K_csc)
                # A^T rows feed AG(A^T) straight away
                nc.sync.dma_start(agat_in[r0:r0 + P, :], atb[rt][:])
                nc.vector.tensor_reduce(out=stat[rt][:, 13:14], in_=atb[rt][:],
                                        axis=AX.X, op=OP.max)

            # ---- phase A: normalize + transpose (xnt scope closes after C) ----
            xp_cm = tc.tile_pool(name="xpool", bufs=1)
            xp = xp_cm.__enter__()
            xnt = [xp.tile([P, N], F32, tag=f"xnt{d}", name=f"xnt{d}") for d in range(DT)]
            xnt_own = [xp.tile([P, ROWS], F32, tag=f"xnto{d}", name=f"xnto{d}") for d in range(DT)]
            with tc.tile_pool(name="pha", bufs=3) as pa, \
                 tc.tile_pool(name="pha_ps", bufs=4, space="PSUM") as paps:
                for t in range(KT):
                    et = pa.tile([P, D], F32, tag="emb_t")
                    nc.sync.dma_start(et[:], emb_in[t * P:(t + 1) * P, :])
                    sq = pa.tile([P, D], F32, tag="sq_t")
                    nc.vector.tensor_tensor(out=sq[:], in0=et[:], in1=et[:], op=OP.mult)
                    ss = sp.tile([P, 8], F32, tag=f"ss{t % 4}", name=f"ss{t}")
                    nc.vector.tensor_reduce(out=ss[:, 0:1], in_=sq[:], axis=AX.X, op=OP.add)
                    rn = _rsqrt(nc, ss, 0)
                    xt = pa.tile([P, D], F32, tag="xn_t")
                    nc.scalar.activation(xt[:], et[:], ACT.Copy, scale=rn)
                    nc.vector.tensor_copy(embb[t][:], et[:])
                    for d in range(DT):
                        pt = paps.tile([P, P], F32, tag="tr_ps", space="PSUM")
                        nc.tensor.transpose(pt[:], xt[:, d * P:(d + 1) * P], i128[:])
                        nc.scalar.copy(xnt[d][:, t * P:(t + 1) * P], pt[:])
                for rt in range(RT):
                    et = pa.tile([P, D], F32, tag="emb_t")
                    nc.sync.dma_start(et[:], emb_own_in[rt * P:(rt + 1) * P, :])
                    sq = pa.tile([P, D], F32, tag="sq_t")
                    nc.vector.tensor_tensor(out=sq[:], in0=et[:], in1=et[:], op=OP.mult)
                    so = sp.tile([P, 8], F32, tag=f"sso{rt}", name=f"sso{rt}")
                    nc.vector.tensor_reduce(out=so[:, 0:1], in_=sq[:], axis=AX.X, op=OP.add)
                    rn = _rsqrt(nc, so, 0)
                    xt = pa.tile([P, D], F32, tag="xn_t")
                    nc.scalar.activation(xt[:], et[:], ACT.Copy, scale=rn)
                    for d in range(DT):
                        pt = paps.tile([P, P], F32, tag="tr_ps", space="PSUM")
                        nc.tensor.transpose(pt[:], xt[:, d * P:(d + 1) * P], i128[:])
                        nc.scalar.copy(xnt_own[d][:, rt * P:(rt + 1) * P], pt[:])

            # ---- C rows (fp32) -> fraw ----
            with tc.tile_pool(name="c_ps", bufs=4, space="PSUM") as cps:
                for rt in range(RT):
                    for j in range(4):
                        pt = cps.tile([P, D], F32, tag="c_ps", space="PSUM")
                        for d in range(DT):
                            nc.tensor.matmul(
                                pt[:], xnt_own[d][:, rt * P:(rt + 1) * P],
                                xnt[d][:, j * D:(j + 1) * D],
                                start=(d == 0), stop=(d == DT - 1))
                        nc.scalar.copy(fraw[rt][:, j * D:(j + 1) * D], pt[:])
            xp_cm.__exit__(None, None, None)

            # ---- A[:, own] = transpose(A^T[own, :]) on PE (bf16) ----
            acol = [pp.tile([P, ROWS], BF16, tag=f"acol{k}", name=f"acol{k}") for k in range(KT)]
            with tc.tile_pool(name="at_ps", bufs=4, space="PSUM") as atps:
                for k in range(KT):
                    for rt in range(RT):
                        pt = atps.tile([P, P], BF16, tag="at_ps", space="PSUM")
                        nc.tensor.transpose(pt[:], atb[rt][:, k * P:(k + 1) * P], i128b[:])
                        nc.scalar.copy(acol[k][:, rt * P:(rt + 1) * P], pt[:])

            # ---- P column sums (independent of softmax) -> ar_in row 1 ----
            with tc.tile_pool(name="csp_ps", bufs=2, space="PSUM") as csps0:
                for j in range(4):
                    pt2 = csps0.tile([1, D], F32, tag="csp_ps", space="PSUM")
                    for rt in range(RT):
                        nc.tensor.matmul(pt2[:], ones_b[:], pbf[rt][:, j * D:(j + 1) * D],
                                         start=(rt == 0), stop=(rt == RT - 1))
                    row2 = sp.tile([1, D], F32, tag=f"csc_{j}", name=f"cscc_{j}")
                    nc.scalar.copy(row2[:], pt2[:])
                    nc.sync.dma_start(ar_in[1, j * D:(j + 1) * D], row2[:])

            # ---- masked softmax ----
            bc2_cm = tc.tile_pool(name="bc2", bufs=1)
            bc2 = bc2_cm.__enter__()
            bcs_cm = tc.tile_pool(name="bcs", bufs=2)
            bcs = bcs_cm.__enter__()
            dmsp_cm = tc.tile_pool(name="dmsp", bufs=1)
            dmsp = dmsp_cm.__enter__()
            fs = [bc2.tile([P, N], F32, tag=f"fs{rt}", name=f"fs{rt}") for rt in range(RT)]
            supp = [bc2.tile([P, N], F32, tag=f"supp{rt}", name=f"supp{rt}") for rt in range(RT)]
            sbf = [bc2.tile([P, N], BF16, tag=f"sbf{rt}", name=f"sbf{rt}") for rt in range(RT)]
            pmat = [bc2.tile([P, N], F32, tag=f"pmat{rt}", name=f"pmat{rt}") for rt in range(RT)]
            for rt in range(RT):
                z = zs[rt]
                nc.vector.tensor_copy(pmat[rt][:], pbf[rt][:])
                nc.vector.tensor_scalar_min(supp[rt][:], pmat[rt][:], 1.0)
                nc.vector.tensor_scalar_add(fraw[rt][:], fraw[rt][:], 4.0)
                scr = bcs.tile([P, N], F32, tag="scr")
                nc.vector.tensor_tensor(out=scr[:], in0=fraw[rt][:],
                                        in1=supp[rt][:], op=OP.mult)
                nc.vector.tensor_reduce(out=z[:, 0:1], in_=scr[:], axis=AX.X, op=OP.max)
                nc.vector.tensor_scalar_mul(z[:, 1:2], z[:, 0:1], -1.0)
                nc.scalar.activation(fraw[rt][:], fraw[rt][:], ACT.Exp, bias=z[:, 1:2])
                nc.vector.scalar_tensor_tensor(
                    out=fs[rt][:], in0=fraw[rt][:], scalar=1.0, in1=pmat[rt][:],
                    op0=OP.mult, op1=OP.mult, accum_out=z[:, 2:3])
                nc.vector.tensor_scalar_max(z[:, 3:4], z[:, 2:3], 1e-30)
                nc.vector.reciprocal(z[:, 4:5], z[:, 3:4])
                nc.scalar.activation(fraw[rt][:], fraw[rt][:], ACT.Copy, scale=z[:, 4:5])
                nc.scalar.activation(fs[rt][:], fs[rt][:], ACT.Copy, scale=z[:, 4:5])
                # FS bf16 -> AG(FS) + A2A payloads
                nc.vector.tensor_copy(sbf[rt][:], fs[rt][:])
                nc.sync.dma_start(agfs_in[rt * P:(rt + 1) * P, :], sbf[rt][:])
                for js in range(NCORES):
                    nc.sync.dma_start(a2a_in[js, rt * P:(rt + 1) * P, :],
                                      sbf[rt][:, js * ROWS:(js + 1) * ROWS])

            # ---- column sums -> ar_in ----
            with tc.tile_pool(name="cs_ps", bufs=4, space="PSUM") as csps:
                for j in range(4):
                    pt = csps.tile([1, D], F32, tag="cs_ps", space="PSUM")
                    for rt in range(RT):
                        nc.tensor.matmul(pt[:], ones_f[:], fs[rt][:, j * D:(j + 1) * D],
                                         start=(rt == 0), stop=(rt == RT - 1))
                    row = sp.tile([1, D], F32, tag=f"csr_{j}", name=f"csr_{j}")
                    nc.scalar.copy(row[:], pt[:])
                    nc.sync.dma_start(ar_in[0, j * D:(j + 1) * D], row[:])

            # ---- collectives (explicitly chained to fix queue order) ----
            cc_at = nc.gpsimd.collective_compute("AllGather", OP.bypass, replica_groups=rg,
                                         ins=[agat_in[:].opt()], outs=[agat_out[:].opt()])
            cc_fs = nc.gpsimd.collective_compute("AllGather", OP.bypass, replica_groups=rg,
                                         ins=[agfs_in[:].opt()], outs=[agfs_out[:].opt()])
            cc_ar = nc.gpsimd.collective_compute("AllReduce", OP.add, replica_groups=rg,
                                         ins=[ar_in[:].opt()], outs=[ar_out[:].opt()])
            cc_a2a = nc.gpsimd.collective_compute("AllToAll", OP.bypass, replica_groups=rg,
                                         ins=[a2a_in[:].opt()], outs=[a2a_out[:].opt()])
            add_dep_helper(cc_fs.ins, cc_at.ins, reason="cc order")
            add_dep_helper(cc_ar.ins, cc_fs.ins, reason="cc order")
            add_dep_helper(cc_a2a.ins, cc_ar.ins, reason="cc order")

            # ---- fitness per edge (union gather from Fraw) ----
            for rt in range(RT):
                r0 = rt * P
                ut = sp.tile([P, K_csr], U16, tag=f"ut{rt}", name=f"ut{rt}")
                nc.sync.dma_start(ut[:], uni_in[r0:r0 + P, :])
                g = bcs.tile([P, 16 * K_csr], F32, tag="gath")
                nc.gpsimd.indirect_copy(g[:], fraw[rt][:], ut[:], True)
                gv = g[:].rearrange("p (b s) -> p s b", b=16)
                mv = m16[:].unsqueeze(1).to_broadcast([P, K_csr, 16])
                g2 = bcs.tile([P, 16 * K_csr], F32, tag="gath2")
                g2v = g2[:].rearrange("p (b s) -> p s b", b=16)
                nc.vector.tensor_tensor(out=g2v, in0=gv, in1=mv, op=OP.mult)
                ft = sp.tile([P, K_csr], F32, tag=f"ft{rt}", name=f"ft{rt}")
                nc.vector.tensor_reduce(out=ft[:], in_=g2v, axis=AX.X, op=OP.add)
                nc.sync.dma_start(fit_out[r0:r0 + P, :], ft[:])



            # ---- scores (identical on every core) ----
            num_row = sp.tile([1, N], F32, tag="num_row")
            cnt_row = sp.tile([1, N], F32, tag="cnt_row")
            nc.gpsimd.dma_start(num_row[:], ar_out[0, :])
            nc.gpsimd.dma_start(cnt_row[:], ar_out[1, :])
            sc_row = sp.tile([1, N], F32, tag="sc_row")
            nc.vector.tensor_scalar_max(sc_row[:], cnt_row[:], 1.0)
            nc.vector.reciprocal(sc_row[:], sc_row[:])
            nc.vector.tensor_tensor(out=sc_row[:], in0=sc_row[:], in1=num_row[:], op=OP.mult)
            nc.gpsimd.dma_start(scr_sc[:], sc_row[:])
            scb = bcs.tile([P, N], F32, tag="bcast", bufs=1)
            nc.gpsimd.dma_start(scb[:], bass.AP(scr_sc.tensor, 0, [[0, P], [1, N]]))

            # ---- cluster mask ----
            dcj_tile = sp.tile([P, RT], F32, tag="dcjs")
            nc.sync.dma_start(dcj_tile[:], dcj_s[:, :])
            for rt in range(RT):
                st = stat[rt]
                dms = dmsp.tile([P, N], F32, tag="dms", name=f"dmsa{rt}")
                nc.vector.tensor_scalar(dms[:], jidx[:], dcj_tile[:, rt:rt + 1],
                                        None, op0=OP.is_equal)
                scr = bcs.tile([P, N], F32, tag="scr")
                # scores_own via diag extraction; m_s via masked rowmax
                nc.vector.scalar_tensor_tensor(
                    out=scr[:], in0=scb[:], scalar=1.0, in1=dms[:],
                    op0=OP.mult, op1=OP.mult, accum_out=st[:, 12:13])
                scr2 = bcs.tile([P, N], F32, tag="scr")
                nc.vector.tensor_tensor(out=scr2[:], in0=scb[:], in1=supp[rt][:], op=OP.mult)
                nc.vector.tensor_reduce(out=st[:, 0:1], in_=scr2[:], axis=AX.X, op=OP.max)
                nc.vector.tensor_reduce(out=st[:, 1:2], in_=supp[rt][:], axis=AX.X, op=OP.max)
                nc.vector.tensor_tensor(out=st[:, 2:3], in0=st[:, 12:13],
                                        in1=st[:, 0:1], op=OP.is_ge)
                nc.vector.tensor_tensor(out=st[:, 3:4], in0=st[:, 2:3],
                                        in1=st[:, 1:2], op=OP.mult)   # cm
                nc.sync.dma_start(bass.AP(agcm_in.tensor, rt * P, [[1, P]]), st[:, 3:4])
                nc.sync.dma_start(bass.AP(scr_cm.tensor, rt * P, [[1, P]]), st[:, 3:4])

            cc_cm = nc.gpsimd.collective_compute("AllGather", OP.bypass, replica_groups=rg,
                                         ins=[agcm_in[:].opt()], outs=[agcm_out[:].opt()])
            add_dep_helper(cc_cm.ins, cc_a2a.ins, reason="cc order")
            cmb = bcs.tile([P, N], F32, tag="bcast", bufs=1)
            nc.gpsimd.dma_start(cmb[:], bass.AP(agcm_out.tensor, 0, [[0, P], [1, N]]))

            # ---- in_node / diagv / col_mask / S rows ----
            nc.vector.memset(cmk_all[:], 0.0)
            for rt in range(RT):
                st = stat[rt]
                scr = bcs.tile([P, N], F32, tag="scr")
                nc.vector.tensor_tensor(out=scr[:], in0=cmb[:], in1=supp[rt][:], op=OP.mult)
                nc.vector.tensor_reduce(out=st[:, 4:5], in_=scr[:], axis=AX.X, op=OP.max)
                nc.vector.tensor_scalar(st[:, 5:6], st[:, 13:14], 0.0, None, op0=OP.is_gt)  # has_in
                nc.vector.tensor_tensor(out=st[:, 6:7], in0=st[:, 3:4], in1=st[:, 5:6], op=OP.mult)
                nc.vector.tensor_tensor(out=st[:, 7:8], in0=st[:, 4:5], in1=st[:, 6:7], op=OP.max)
                nc.vector.tensor_scalar(st[:, 8:9], st[:, 7:8], 0.0, None, op0=OP.is_gt)
                nc.vector.tensor_scalar(st[:, 9:10], st[:, 8:9], -1.0, 1.0,
                                        op0=OP.mult, op1=OP.add)          # non_in
                nc.vector.tensor_tensor(out=st[:, 10:11], in0=st[:, 3:4],
                                        in1=st[:, 9:10], op=OP.add)       # diagv
                nc.vector.tensor_tensor(out=st[:, 11:12], in0=st[:, 3:4],
                                        in1=st[:, 9:10], op=OP.max)       # col_mask
                nc.sync.dma_start(cmask_out[:, rt:rt + 1], st[:, 11:12])
                nc.sync.dma_start(bass.AP(agdv_in.tensor, rt * P, [[1, P]]), st[:, 10:11])
                nc.sync.dma_start(bass.AP(scr_cm.tensor, ROWS + rt * P, [[1, P]]), st[:, 10:11])
                for i in range(KT // RT):
                    nc.vector.tensor_copy(cmk_all[:, rt + i * RT:rt + i * RT + 1],
                                          st[:, 11:12])
                # S rows f32 (output)
                dms2 = dmsp.tile([P, N], F32, tag="dms", name=f"dmsb{rt}")
                nc.vector.tensor_scalar(dms2[:], jidx[:], dcj_tile[:, rt:rt + 1],
                                        None, op0=OP.is_equal)
                nc.vector.tensor_tensor(out=fs[rt][:], in0=fs[rt][:], in1=cmb[:], op=OP.mult)
                nc.vector.scalar_tensor_tensor(
                    out=fs[rt][:], in0=dms2[:], scalar=st[:, 10:11], in1=fs[rt][:],
                    op0=OP.mult, op1=OP.add)
                nc.sync.dma_start(s_out[rt * P:(rt + 1) * P, :], fs[rt][:])

            dmsp_cm.__exit__(None, None, None)
            bcs_cm.__exit__(None, None, None)
            bc2_cm.__exit__(None, None, None)
            bc1_cm.__exit__(None, None, None)

            # ================= phase D =================
            with tc.tile_pool(name="dp", bufs=1) as dp, \
                 tc.tile_pool(name="dps", bufs=4) as dps:
                cc_fs = nc.gpsimd.collective_compute("AllGather", OP.bypass, replica_groups=rg,
                                             ins=[agfs_in[:].opt()], outs=[agfs_out[:].opt()])
                cc_dv = nc.gpsimd.collective_compute("AllGather", OP.bypass, replica_groups=rg,
                                             ins=[agdv_in[:].opt()], outs=[agdv_out[:].opt()])
                add_dep_helper(cc_fs.ins, cc_cm.ins, reason="cc order")
                add_dep_helper(cc_dv.ins, cc_fs.ins, reason="cc order")
                fscol = [dp.tile([P, ROWS], BF16, tag=f"fscol{k}", name=f"fscol{k}") for k in range(KT)]
                asb = [dp.tile([P, ROWS], BF16, tag=f"asb{k}", name=f"asb{k}") for k in range(KT)]
                fs_v = bass.AP(a2a_out.tensor, 0, [[ROWS, N], [1, ROWS]])
                for k in range(KT):
                    nc.sync.dma_start(fscol[k][:], fs_v[k * P:(k + 1) * P, :])

                dcje_t = sp.tile([P, KT], F32, tag="dcje")
                nc.sync.dma_start(dcje_t[:], dcj_e[:, :])
                cm_ownb = dp.tile([P, ROWS], F32, tag="cm_ownb")
                dv_ownb = dp.tile([P, ROWS], F32, tag="dv_ownb")
                cm_le = sp.tile([P, KT], F32, tag="cm_le")
                dv_le = sp.tile([P, KT], F32, tag="dv_le")

                # ---- mm1: Y1 = A @ FS[:, own] ----
                with tc.tile_pool(name="mm1_ps", bufs=1, space="PSUM") as mmps:
                    ps1 = [mmps.tile([P, ROWS], F32, tag=f"mm1_{i}", name=f"mm1_{i}", space="PSUM")
                           for i in range(8)]
                    for half in range(2):
                        for k in range(KT):
                            atrow = dps.tile([P, N], BF16, tag="atrow")
                            nc.sync.dma_start(atrow[:], agat_out[k * P:(k + 1) * P, :])
                            for i8 in range(8):
                                it = half * 8 + i8
                                nc.tensor.matmul(
                                    ps1[i8][:], atrow[:, it * P:(it + 1) * P], fscol[k][:],
                                    start=(k == 0), stop=(k == KT - 1))
                        if half == 0:
                            # issue late small loads on gpsimd queue (not sync) so
                            # they cannot stall the matmul input stream
                            nc.gpsimd.dma_start(cm_ownb[:], bass.AP(scr_cm.tensor, 0, [[0, P], [1, ROWS]]))
                            nc.gpsimd.dma_start(dv_ownb[:], bass.AP(scr_cm.tensor, ROWS, [[0, P], [1, ROWS]]))
                            nc.gpsimd.dma_start(cm_le[:], bass.AP(agcm_out.tensor, 0, [[1, P], [P, KT]]))
                            nc.gpsimd.dma_start(dv_le[:], bass.AP(agdv_out.tensor, 0, [[1, P], [P, KT]]))
                        for i8 in range(8):
                            it = half * 8 + i8
                            # AS = cm_own*Y1 + A[:,own]*diagv_own
                            t1 = dps.tile([P, ROWS], F32, tag="t1", name=f"t1_{it}")
                            nc.vector.tensor_tensor(out=t1[:], in0=acol[it][:],
                                                    in1=dv_ownb[:], op=OP.mult)
                            t2 = dps.tile([P, ROWS], F32, tag="t2", name=f"t2_{it}")
                            nc.vector.tensor_tensor(out=t2[:], in0=ps1[i8][:],
                                                    in1=cm_ownb[:], op=OP.mult)
                            nc.vector.tensor_tensor(out=asb[it][:], in0=t2[:],
                                                    in1=t1[:], op=OP.add)

                # ---- mm2: Emat = cm[i]*(FS^T @ AS) + diagv[i]*AS ----
                with tc.tile_pool(name="mm2_ps", bufs=1, space="PSUM") as mmps2:
                    ps2 = [mmps2.tile([P, ROWS], F32, tag=f"mm2_{i}", name=f"mm2_{i}", space="PSUM")
                           for i in range(8)]
                    for half in range(2):
                        for k in range(KT):
                            srow = dps.tile([P, N], BF16, tag="srow")
                            nc.sync.dma_start(srow[:], agfs_out[k * P:(k + 1) * P, :])
                            for i8 in range(8):
                                it = half * 8 + i8
                                nc.tensor.matmul(
                                    ps2[i8][:], srow[:, it * P:(it + 1) * P], asb[k][:],
                                    start=(k == 0), stop=(k == KT - 1))
                        for i8 in range(8):
                            it = half * 8 + i8
                            dme = dps.tile([P, ROWS], F32, tag="dme", name=f"dme{it}")
                            nc.vector.tensor_scalar(dme[:], jidx[:, 0:ROWS],
                                                    dcje_t[:, it:it + 1], None, op0=OP.is_equal)
                            t1 = dps.tile([P, ROWS], F32, tag="t1", name=f"t1e_{it}")
                            nc.vector.tensor_scalar(t1[:], asb[it][:], dv_le[:, it:it + 1],
                                                    None, op0=OP.mult)
                            eo = dps.tile([P, ROWS], F32, tag="eo", name=f"eo{it}")
                            nc.vector.scalar_tensor_tensor(
                                out=eo[:], in0=ps2[i8][:], scalar=cm_le[:, it:it + 1],
                                in1=t1[:], op0=OP.mult, op1=OP.add)
                            de = sp.tile([P, 4], F32, tag=f"de{it % 4}", name=f"de{it}")
                            scr_e = dps.tile([P, ROWS], F32, tag="scr_e", name=f"scr_e{it}")
                            nc.vector.scalar_tensor_tensor(
                                out=scr_e[:], in0=eo[:], scalar=1.0, in1=dme[:],
                                op0=OP.mult, op1=OP.mult, accum_out=de[:, 0:1])
                            nc.vector.tensor_scalar(de[:, 1:2], de[:, 0:1], 0.0, None,
                                                    op0=OP.is_equal)
                            nc.vector.tensor_tensor(out=de[:, 2:3], in0=de[:, 1:2],
                                                    in1=cmk_all[:, it:it + 1], op=OP.mult)
                            nc.vector.scalar_tensor_tensor(
                                out=eo[:], in0=dme[:], scalar=de[:, 2:3], in1=eo[:],
                                op0=OP.mult, op1=OP.add)
                            nc.sync.dma_start(emat_out[it * P:(it + 1) * P, :], eo[:])

                # ---- pooled ----
                with tc.tile_pool(name="mm3_ps", bufs=2, space="PSUM") as mmps3:
                    mcol = [dp.tile([P, ROWS], BF16, tag=f"mcol{k}", name=f"mcol{k}") for k in range(KT)]
                    for k in range(KT):
                        dme = dps.tile([P, ROWS], F32, tag="dme", name=f"dmep{k}")
                        nc.vector.tensor_scalar(dme[:], jidx[:, 0:ROWS],
                                                dcje_t[:, k:k + 1], None, op0=OP.is_equal)
                        sc_t = dps.tile([P, ROWS], F32, tag="sc_t", name=f"sc_t{k}")
                        nc.vector.tensor_tensor(out=sc_t[:], in0=fscol[k][:],
                                                in1=cm_ownb[:], op=OP.mult)
                        nc.vector.scalar_tensor_tensor(
                            out=sc_t[:], in0=dme[:], scalar=dv_le[:, k:k + 1], in1=sc_t[:],
                            op0=OP.mult, op1=OP.add)
                        nc.vector.tensor_scalar(mcol[k][:], sc_t[:], 0.0, None, op0=OP.is_gt)
                    for jt in range(RT):
                        pl_ps = mmps3.tile([P, D], F32, tag="pool_ps", space="PSUM")
                        pc_ps = mmps3.tile([P, 2], F32, tag="cnt_ps", space="PSUM")
                        for k in range(KT):
                            nc.tensor.matmul(pl_ps[:], mcol[k][:, jt * P:(jt + 1) * P],
                                             embb[k][:], start=(k == 0), stop=(k == KT - 1))
                            nc.tensor.matmul(pc_ps[:, 0:1], mcol[k][:, jt * P:(jt + 1) * P],
                                             ones_b[:], start=(k == 0), stop=(k == KT - 1))
                        cn = sp.tile([P, 4], F32, tag=f"cn{jt}", name=f"cn{jt}")
                        nc.vector.tensor_scalar_max(cn[:, 0:1], pc_ps[:, 0:1], 1.0)
                        nc.vector.reciprocal(cn[:, 1:2], cn[:, 0:1])
                        nc.vector.tensor_tensor(out=cn[:, 2:3], in0=cn[:, 1:2],
                                                in1=stat[jt][:, 11:12], op=OP.mult)
                        po = dps.tile([P, D], F32, tag="po")
                        nc.scalar.activation(po[:], pl_ps[:], ACT.Copy, scale=cn[:, 2:3])
                        nc.sync.dma_start(pooled_out[jt * P:(jt + 1) * P, :], po[:])

    nc.compile()
    return nc


# --------------------------------------------------------------------------
# host prep / assembly
# --------------------------------------------------------------------------
def _prep(edge_index):
    src = np.asarray(edge_index[0], np.int64)
    dst = np.asarray(edge_index[1], np.int64)
    key = src * N + dst
    uniq, inv, mult = np.unique(key, return_inverse=True, return_counts=True)
    usrc = (uniq // N).astype(np.int64)
    udst = (uniq % N).astype(np.int64)
    Eu = len(uniq)

    deg = np.bincount(usrc, minlength=N)
    K_csr = max(2, int(np.ceil(deg.max() / 2) * 2))
    row_start = np.zeros(N + 1, np.int64)
    np.cumsum(deg, out=row_start[1:])
    pos = np.arange(Eu) - row_start[usrc]

    dstM = np.full((N, K_csr), -1, np.int64)
    dstM[usrc, pos] = udst
    multM = np.zeros((N, K_csr), np.float32)
    multM[usrc, pos] = mult

    h0 = np.where((dstM >= 0) & (dstM < N // 2), dstM, -1).astype(np.int16)
    h1 = np.where(dstM >= N // 2, dstM - N // 2, -1).astype(np.int16)

    dst_clip = np.maximum(dstM, 0).astype(np.uint16)
    uni = np.zeros((N, K_csr), np.uint16)
    wrap = (np.arange(K_csr)[None, :] * 16 + np.arange(16)[:, None]).reshape(-1)
    for g in range(N // 16):
        ul = dst_clip[g * 16:(g + 1) * 16].reshape(-1)
        uni[g * 16:(g + 1) * 16] = ul[wrap].reshape(16, K_csr)

    order2 = np.lexsort((usrc, udst))
    csrc = usrc[order2]
    cdst = udst[order2]
    cmult = mult[order2]
    ideg = np.bincount(cdst, minlength=N)
    K_csc = max(2, int(np.ceil(ideg.max() / 2) * 2))
    cstart = np.zeros(N + 1, np.int64)
    np.cumsum(ideg, out=cstart[1:])
    cpos = np.arange(Eu) - cstart[cdst]
    srcM = np.full((N, K_csc), -1, np.int64)
    srcM[cdst, cpos] = csrc
    cmultM = np.zeros((N, K_csc), np.float32)
    cmultM[cdst, cpos] = cmult
    c0 = np.where((srcM >= 0) & (srcM < N // 2), srcM, -1).astype(np.int16)
    c1 = np.where(srcM >= N // 2, srcM - N // 2, -1).astype(np.int16)

    in_maps = []
    rows_p = np.arange(P)
    for c in range(NCORES):
        base = c * ROWS
        sl = slice(base, base + ROWS)
        dcj_sv = (base + P * np.arange(RT)[None, :] + rows_p[:, None]).astype(np.float32)
        dcj_ev = (P * np.arange(KT)[None, :] + rows_p[:, None] - base).astype(np.float32)
        in_maps.append({
            "csr_h0": h0[sl].copy(), "csr_h1": h1[sl].copy(),
            "csr_mult": multM[sl].astype(ml_dtypes.bfloat16),
            "uni_idx": uni[sl].copy(),
            "csc_h0": c0[sl].copy(), "csc_h1": c1[sl].copy(),
            "csc_mult": cmultM[sl].astype(ml_dtypes.bfloat16),
            "diag_cj_s": np.ascontiguousarray(dcj_sv),
            "diag_cj_e": np.ascontiguousarray(dcj_ev),
        })
    fit_row = usrc[inv]
    fit_slot = pos[inv]
    return in_maps, K_csr, K_csc, fit_row, fit_slot


def prepare(embedding, edge_index):
    emb = np.ascontiguousarray(np.asarray(embedding, np.float32))
    ei = np.asarray(edge_index)
    in_maps, K_csr, K_csc, fit_row, fit_slot = _prep(ei)
    for c in range(NCORES):
        in_maps[c]["emb"] = emb
        in_maps[c]["emb_own"] = np.ascontiguousarray(emb[c * ROWS:(c + 1) * ROWS])
    ck = (K_csr, K_csc)
    if ck not in _cache:
        _cache[ck] = _build(K_csr, K_csc)
    return _cache[ck], in_maps, fit_row, fit_slot


def assemble(results, fit_row, fit_slot):
    r = results
    S = np.concatenate([np.asarray(r[c]["s_out"]) for c in range(NCORES)], axis=0)
    Emat = np.concatenate([np.asarray(r[c]["emat_out"]) for c in range(NCORES)], axis=1)
    pooled = np.concatenate([np.asarray(r[c]["pooled_out"]) for c in range(NCORES)], axis=0)
    cmask = np.concatenate(
        [np.asarray(r[c]["cmask_out"]).T.reshape(-1) for c in range(NCORES)]) > 0.5
    fit_all = np.concatenate([np.asarray(r[c]["fit_out"]) for c in range(NCORES)], axis=0)
    fitness = np.ascontiguousarray(fit_all[fit_row, fit_slot].astype(np.float32))
    return (np.ascontiguousarray(pooled.astype(np.float32)),
            np.ascontiguousarray(Emat.astype(np.float32)),
            np.ascontiguousarray(S.astype(np.float32)),
            fitness, cmask)


def kernel(embedding, edge_index):
    nc, in_maps, fit_row, fit_slot = prepare(embedding, edge_index)
    res = run_bass_kernel_spmd(nc, in_maps, core_ids=list(range(NCORES)))
    return assemble(res.results, fit_row, fit_slot)


# revision 23
# speedup vs baseline: 1.0699x; 1.0699x over previous
"""Adaptive graph pooling (gnn_message_passing) on 8 TRN2 NeuronCores.

Sharding: nodes 256-per-core.  Host work is limited to sharding / index prep
(CSR/CSC bucketing of the edge list) and output assembly.

v2 pipeline (collectives start as early as their inputs allow; S-dependent
matmuls are decomposed so the big AllGathers ship cm-independent tensors):

  S = FS .* cm[col] + diag(diagv)          (FS = multiplicity-weighted fitness)
  AS[:,own]   = cm[own]*(A @ FS[:,own]) + A[:,own]*diagv[own]
  Emat[:,own] = cm[row] * (FS^T @ AS) + diagv[row] * AS

  collective order: AG(A^T) | AllReduce(colsums) | A2A(FS,A cols) | AG(cm)
                    | AG(FS) | AG(diagv)
  mm1 = A @ FS[:,own] runs while AG(FS)/AG(cm) are on the wire.
"""
import sys
if '/opt/trn_rl_repo' not in sys.path:
    sys.path.insert(0, '/opt/trn_rl_repo')

import numpy as np
import ml_dtypes

import concourse.bass as bass
import concourse.tile as tile
from concourse.tile import add_dep_helper
from concourse import bacc, mybir, library_config
from concourse.bass_utils import run_bass_kernel_spmd

F32 = mybir.dt.float32
BF16 = mybir.dt.bfloat16
I16 = mybir.dt.int16
U16 = mybir.dt.uint16
AX = mybir.AxisListType
OP = mybir.AluOpType
ACT = mybir.ActivationFunctionType

N = 2048
D = 512
NCORES = 8
ROWS = N // NCORES
P = 128
RT = ROWS // P
KT = N // P
DT = D // P

_cache = {}


def _rsqrt(nc, ss, col):
    """ss[:, col] = sumsq -> returns AP of 1/max(sqrt(ss),1e-12)."""
    c = col
    nc.scalar.activation(ss[:, c + 1:c + 2], ss[:, c:c + 1], ACT.Sqrt)
    nc.vector.tensor_scalar_max(ss[:, c + 2:c + 3], ss[:, c + 1:c + 2], 1e-12)
    nc.vector.reciprocal(ss[:, c + 3:c + 4], ss[:, c + 2:c + 3])
    return ss[:, c + 3:c + 4]


def _build(K_csr, K_csc):
    nc = bacc.Bacc("TRN2", target_bir_lowering=False, debug=False,
                   enable_asserts=False, num_devices=NCORES)

    # ---- I/O ----
    emb_in = nc.dram_tensor("emb", [N, D], F32, kind="ExternalInput").ap()
    emb_own_in = nc.dram_tensor("emb_own", [ROWS, D], F32, kind="ExternalInput").ap()
    csr_h0 = nc.dram_tensor("csr_h0", [ROWS, K_csr], I16, kind="ExternalInput").ap()
    csr_h1 = nc.dram_tensor("csr_h1", [ROWS, K_csr], I16, kind="ExternalInput").ap()
    csr_mult = nc.dram_tensor("csr_mult", [ROWS, K_csr], BF16, kind="ExternalInput").ap()
    uni_in = nc.dram_tensor("uni_idx", [ROWS, K_csr], U16, kind="ExternalInput").ap()
    csc_h0 = nc.dram_tensor("csc_h0", [ROWS, K_csc], I16, kind="ExternalInput").ap()
    csc_h1 = nc.dram_tensor("csc_h1", [ROWS, K_csc], I16, kind="ExternalInput").ap()
    csc_mult = nc.dram_tensor("csc_mult", [ROWS, K_csc], BF16, kind="ExternalInput").ap()
    dcj_s = nc.dram_tensor("diag_cj_s", [P, RT], F32, kind="ExternalInput").ap()
    dcj_e = nc.dram_tensor("diag_cj_e", [P, KT], F32, kind="ExternalInput").ap()

    s_out = nc.dram_tensor("s_out", [ROWS, N], F32, kind="ExternalOutput").ap()
    emat_out = nc.dram_tensor("emat_out", [N, ROWS], F32, kind="ExternalOutput").ap()
    pooled_out = nc.dram_tensor("pooled_out", [ROWS, D], F32, kind="ExternalOutput").ap()
    fit_out = nc.dram_tensor("fit_out", [ROWS, K_csr], F32, kind="ExternalOutput").ap()
    cmask_out = nc.dram_tensor("cmask_out", [P, RT], F32, kind="ExternalOutput").ap()

    # ---- collective bounces + local scratch (internal DRAM) ----
    agat_in = nc.dram_tensor("agat_in", [ROWS, N], BF16).ap()
    agat_out = nc.dram_tensor("agat_out", [N, N], BF16, addr_space="Shared").ap()
    ar_in = nc.dram_tensor("ar_in", [2, N], F32).ap()
    ar_out = nc.dram_tensor("ar_out", [2, N], F32, addr_space="Shared").ap()
    a2a_in = nc.dram_tensor("a2a_in", [NCORES, ROWS, ROWS], BF16).ap()
    a2a_out = nc.dram_tensor("a2a_out", [NCORES, ROWS, ROWS], BF16).ap()
    agcm_in = nc.dram_tensor("agcm_in", [ROWS], F32).ap()
    agcm_out = nc.dram_tensor("agcm_out", [N], F32, addr_space="Shared").ap()
    agfs_in = nc.dram_tensor("agfs_in", [ROWS, N], BF16).ap()
    agfs_out = nc.dram_tensor("agfs_out", [N, N], BF16, addr_space="Shared").ap()
    agdv_in = nc.dram_tensor("agdv_in", [ROWS], F32).ap()
    agdv_out = nc.dram_tensor("agdv_out", [N], F32, addr_space="Shared").ap()
    scr_cm = nc.dram_tensor("scr_cm", [2, ROWS], F32).ap()     # cm_own | diagv_own
    warm_in = nc.dram_tensor("warm_in", [64], F32).ap()
    warm_out = nc.dram_tensor("warm_out", [512], F32, addr_space="Shared").ap()
    scr_sc = nc.dram_tensor("scr_sc", [N], F32).ap()           # scores roundtrip

    # ---- constants ----
    jidx_np = np.broadcast_to(np.arange(N, dtype=np.float32), (P, N)).copy()
    i128_np = np.eye(P, dtype=np.float32)
    m16_np = np.zeros((P, 16), np.float32)
    m16_np[np.arange(P), np.arange(P) % 16] = 1.0
    jidx_c = nc.inline_tensor(jidx_np, "jidx_c").ap()
    i128_c = nc.inline_tensor(i128_np, "i128_c").ap()
    i128b_c = nc.inline_tensor(i128_np.astype(ml_dtypes.bfloat16), "i128b_c").ap()
    m16_c = nc.inline_tensor(m16_np, "m16_c").ap()

    rg = [list(range(NCORES))]

    with tile.TileContext(nc) as tc:
        with tc.tile_pool(name="const", bufs=1) as cpool, \
             tc.tile_pool(name="persist", bufs=1) as pp, \
             tc.tile_pool(name="small", bufs=1) as sp:

            nc.gpsimd.load_library(library_config.local_scatter)
            wz = sp.tile([1, 64], F32, tag="wz")
            nc.vector.memset(wz[:], 0.0)
            nc.sync.dma_start(warm_in[:], wz[:])
            nc.gpsimd.collective_compute("AllGather", OP.bypass, replica_groups=rg,
                                         ins=[warm_in[:].opt()], outs=[warm_out[:].opt()])

            jidx = cpool.tile([P, N], F32)
            i128 = cpool.tile([P, P], F32)
            i128b = cpool.tile([P, P], BF16)
            m16 = cpool.tile([P, 16], F32)
            ones_f = cpool.tile([P, 1], F32)
            ones_b = cpool.tile([P, 1], BF16)
            nc.sync.dma_start(jidx[:], jidx_c[:])
            nc.sync.dma_start(i128[:], i128_c[:])
            nc.sync.dma_start(i128b[:], i128b_c[:])
            nc.sync.dma_start(m16[:], m16_c[:])
            nc.vector.memset(ones_f[:], 1.0)
            nc.vector.memset(ones_b[:], 1.0)

            embb = [pp.tile([P, D], BF16, tag=f"embb{t}", name=f"embb{t}") for t in range(KT)]
            stat = [sp.tile([P, 16], F32, tag=f"stat{rt}", name=f"stat{rt}") for rt in range(RT)]
            zs = [sp.tile([P, 8], F32, tag=f"zs{rt}", name=f"zs{rt}") for rt in range(RT)]
            cmk_all = sp.tile([P, KT], F32, tag="cmk_all")

            bc1_cm = tc.tile_pool(name="bc1", bufs=1)
            bc1 = bc1_cm.__enter__()
            fraw = [bc1.tile([P, N], F32, tag=f"fraw{rt}", name=f"fraw{rt}") for rt in range(RT)]
            pbf = [bc1.tile([P, N], BF16, tag=f"pbf{rt}", name=f"pbf{rt}") for rt in range(RT)]
            atb = [bc1.tile([P, N], BF16, tag=f"atb{rt}", name=f"atb{rt}") for rt in range(RT)]

            # ---- edge scatters (gpsimd; independent, start immediately) ----
            for rt in range(RT):
                r0 = rt * P
                ih0 = sp.tile([P, K_csr], I16, tag=f"ih0{rt}", name=f"ih0{rt}")
                ih1 = sp.tile([P, K_csr], I16, tag=f"ih1{rt}", name=f"ih1{rt}")
                imu = sp.tile([P, K_csr], BF16, tag=f"imu{rt}", name=f"imu{rt}")
                nc.sync.dma_start(ih0[:], csr_h0[r0:r0 + P, :])
                nc.sync.dma_start(ih1[:], csr_h1[r0:r0 + P, :])
                nc.sync.dma_start(imu[:], csr_mult[r0:r0 + P, :])
                nc.gpsimd.local_scatter(out_ap=pbf[rt][:, 0:N // 2], data_ap=imu[:],
                                        idxs_ap=ih0[:], channels=P,
                                        num_elems=N // 2, num_idxs=K_csr)
                nc.gpsimd.local_scatter(out_ap=pbf[rt][:, N // 2:N], data_ap=imu[:],
                                        idxs_ap=ih1[:], channels=P,
                                        num_elems=N // 2, num_idxs=K_csr)
                ch0 = sp.tile([P, K_csc], I16, tag=f"ch0{rt}", name=f"ch0{rt}")
                ch1 = sp.tile([P, K_csc], I16, tag=f"ch1{rt}", name=f"ch1{rt}")
                cmu = sp.tile([P, K_csc], BF16, tag=f"cmu{rt}", name=f"cmu{rt}")
                nc.sync.dma_start(ch0[:], csc_h0[r0:r0 + P, :])
                nc.sync.dma_start(ch1[:], csc_h1[r0:r0 + P, :])
                nc.sync.dma_start(cmu[:], csc_mult[r0:r0 + P, :])
                nc.gpsimd.local_scatter(out_ap=atb[rt][:, 0:N // 2], data_ap=cmu[:],
                                        idxs_ap=ch0[:], channels=P,
                                        num_elems=N // 2, num_idxs=K_csc)
                nc.gpsimd.local_scatter(out_ap=atb[rt][:, N // 2:N], data_ap=cmu[:],
                                        idxs_ap=ch1[:], channels=P,
                                        num_elems=N // 2, num_idxs=K_csc)
                # A^T rows feed AG(A^T) straight away
                nc.sync.dma_start(agat_in[r0:r0 + P, :], atb[rt][:])
                nc.vector.tensor_reduce(out=stat[rt][:, 13:14], in_=atb[rt][:],
                                        axis=AX.X, op=OP.max)

            # ---- phase A: normalize + transpose (xnt scope closes after C) ----
            xp_cm = tc.tile_pool(name="xpool", bufs=1)
            xp = xp_cm.__enter__()
            xnt = [xp.tile([P, N], F32, tag=f"xnt{d}", name=f"xnt{d}") for d in range(DT)]
            xnt_own = [xp.tile([P, ROWS], F32, tag=f"xnto{d}", name=f"xnto{d}") for d in range(DT)]
            with tc.tile_pool(name="pha", bufs=3) as pa, \
                 tc.tile_pool(name="pha_ps", bufs=4, space="PSUM") as paps:
                for t in range(KT):
                    et = pa.tile([P, D], F32, tag="emb_t")
                    nc.sync.dma_start(et[:], emb_in[t * P:(t + 1) * P, :])
                    sq = pa.tile([P, D], F32, tag="sq_t")
                    nc.vector.tensor_tensor(out=sq[:], in0=et[:], in1=et[:], op=OP.mult)
                    ss = sp.tile([P, 8], F32, tag=f"ss{t % 4}", name=f"ss{t}")
                    nc.vector.tensor_reduce(out=ss[:, 0:1], in_=sq[:], axis=AX.X, op=OP.add)
                    rn = _rsqrt(nc, ss, 0)
                    xt = pa.tile([P, D], F32, tag="xn_t")
                    nc.scalar.activation(xt[:], et[:], ACT.Copy, scale=rn)
                    nc.vector.tensor_copy(embb[t][:], et[:])
                    for d in range(DT):
                        pt = paps.tile([P, P], F32, tag="tr_ps", space="PSUM")
                        nc.tensor.transpose(pt[:], xt[:, d * P:(d + 1) * P], i128[:])
                        nc.scalar.copy(xnt[d][:, t * P:(t + 1) * P], pt[:])
                for rt in range(RT):
                    et = pa.tile([P, D], F32, tag="emb_t")
                    nc.sync.dma_start(et[:], emb_own_in[rt * P:(rt + 1) * P, :])
                    sq = pa.tile([P, D], F32, tag="sq_t")
                    nc.vector.tensor_tensor(out=sq[:], in0=et[:], in1=et[:], op=OP.mult)
                    so = sp.tile([P, 8], F32, tag=f"sso{rt}", name=f"sso{rt}")
                    nc.vector.tensor_reduce(out=so[:, 0:1], in_=sq[:], axis=AX.X, op=OP.add)
                    rn = _rsqrt(nc, so, 0)
                    xt = pa.tile([P, D], F32, tag="xn_t")
                    nc.scalar.activation(xt[:], et[:], ACT.Copy, scale=rn)
                    for d in range(DT):
                        pt = paps.tile([P, P], F32, tag="tr_ps", space="PSUM")
                        nc.tensor.transpose(pt[:], xt[:, d * P:(d + 1) * P], i128[:])
                        nc.scalar.copy(xnt_own[d][:, rt * P:(rt + 1) * P], pt[:])

            # ---- C rows (fp32) -> fraw ----
            with tc.tile_pool(name="c_ps", bufs=4, space="PSUM") as cps:
                for rt in range(RT):
                    for j in range(4):
                        pt = cps.tile([P, D], F32, tag="c_ps", space="PSUM")
                        for d in range(DT):
                            nc.tensor.matmul(
                                pt[:], xnt_own[d][:, rt * P:(rt + 1) * P],
                                xnt[d][:, j * D:(j + 1) * D],
                                start=(d == 0), stop=(d == DT - 1))
                        nc.scalar.copy(fraw[rt][:, j * D:(j + 1) * D], pt[:])
            xp_cm.__exit__(None, None, None)

            # ---- A[:, own] = transpose(A^T[own, :]) on PE (bf16) ----
            acol = [pp.tile([P, ROWS], BF16, tag=f"acol{k}", name=f"acol{k}") for k in range(KT)]
            with tc.tile_pool(name="at_ps", bufs=4, space="PSUM") as atps:
                for k in range(KT):
                    for rt in range(RT):
                        pt = atps.tile([P, P], BF16, tag="at_ps", space="PSUM")
                        nc.tensor.transpose(pt[:], atb[rt][:, k * P:(k + 1) * P], i128b[:])
                        nc.scalar.copy(acol[k][:, rt * P:(rt + 1) * P], pt[:])

            # ---- P column sums (independent of softmax) -> ar_in row 1 ----
            with tc.tile_pool(name="csp_ps", bufs=2, space="PSUM") as csps0:
                for j in range(4):
                    pt2 = csps0.tile([1, D], F32, tag="csp_ps", space="PSUM")
                    for rt in range(RT):
                        nc.tensor.matmul(pt2[:], ones_b[:], pbf[rt][:, j * D:(j + 1) * D],
                                         start=(rt == 0), stop=(rt == RT - 1))
                    row2 = sp.tile([1, D], F32, tag=f"csc_{j}", name=f"cscc_{j}")
                    nc.scalar.copy(row2[:], pt2[:])
                    nc.sync.dma_start(ar_in[1, j * D:(j + 1) * D], row2[:])

            # ---- masked softmax ----
            bc2_cm = tc.tile_pool(name="bc2", bufs=1)
            bc2 = bc2_cm.__enter__()
            bcs_cm = tc.tile_pool(name="bcs", bufs=2)
            bcs = bcs_cm.__enter__()
            dmsp_cm = tc.tile_pool(name="dmsp", bufs=1)
            dmsp = dmsp_cm.__enter__()
            fs = [bc2.tile([P, N], F32, tag=f"fs{rt}", name=f"fs{rt}") for rt in range(RT)]
            supp = [bc2.tile([P, N], F32, tag=f"supp{rt}", name=f"supp{rt}") for rt in range(RT)]
            sbf = [bc2.tile([P, N], BF16, tag=f"sbf{rt}", name=f"sbf{rt}") for rt in range(RT)]
            pmat = [bc2.tile([P, N], F32, tag=f"pmat{rt}", name=f"pmat{rt}") for rt in range(RT)]
            for rt in range(RT):
                z = zs[rt]
                nc.vector.tensor_copy(pmat[rt][:], pbf[rt][:])
                nc.vector.tensor_scalar_min(supp[rt][:], pmat[rt][:], 1.0)
                nc.vector.tensor_scalar_add(fraw[rt][:], fraw[rt][:], 4.0)
                scr = bcs.tile([P, N], F32, tag="scr")
                nc.vector.tensor_tensor(out=scr[:], in0=fraw[rt][:],
                                        in1=supp[rt][:], op=OP.mult)
                nc.vector.tensor_reduce(out=z[:, 0:1], in_=scr[:], axis=AX.X, op=OP.max)
                nc.vector.tensor_scalar_mul(z[:, 1:2], z[:, 0:1], -1.0)
                nc.scalar.activation(fraw[rt][:], fraw[rt][:], ACT.Exp, bias=z[:, 1:2])
                nc.vector.scalar_tensor_tensor(
                    out=fs[rt][:], in0=fraw[rt][:], scalar=1.0, in1=pmat[rt][:],
                    op0=OP.mult, op1=OP.mult, accum_out=z[:, 2:3])
                nc.vector.tensor_scalar_max(z[:, 3:4], z[:, 2:3], 1e-30)
                nc.vector.reciprocal(z[:, 4:5], z[:, 3:4])
                nc.scalar.activation(fraw[rt][:], fraw[rt][:], ACT.Copy, scale=z[:, 4:5])
                nc.scalar.activation(fs[rt][:], fs[rt][:], ACT.Copy, scale=z[:, 4:5])
                # FS bf16 -> AG(FS) + A2A payloads
                nc.vector.tensor_copy(sbf[rt][:], fs[rt][:])
                nc.sync.dma_start(agfs_in[rt * P:(rt + 1) * P, :], sbf[rt][:])
                for js in range(NCORES):
                    nc.sync.dma_start(a2a_in[js, rt * P:(rt + 1) * P, :],
                                      sbf[rt][:, js * ROWS:(js + 1) * ROWS])

            # ---- column sums -> ar_in ----
            with tc.tile_pool(name="cs_ps", bufs=4, space="PSUM") as csps:
                for j in range(4):
                    pt = csps.tile([1, D], F32, tag="cs_ps", space="PSUM")
                    for rt in range(RT):
                        nc.tensor.matmul(pt[:], ones_f[:], fs[rt][:, j * D:(j + 1) * D],
                                         start=(rt == 0), stop=(rt == RT - 1))
                    row = sp.tile([1, D], F32, tag=f"csr_{j}", name=f"csr_{j}")
                    nc.scalar.copy(row[:], pt[:])
                    nc.sync.dma_start(ar_in[0, j * D:(j + 1) * D], row[:])

            # ---- collectives (explicitly chained to fix queue order) ----
            cc_at = nc.gpsimd.collective_compute("AllGather", OP.bypass, replica_groups=rg,
                                         ins=[agat_in[:].opt()], outs=[agat_out[:].opt()])
            cc_fs = nc.gpsimd.collective_compute("AllGather", OP.bypass, replica_groups=rg,
                                         ins=[agfs_in[:].opt()], outs=[agfs_out[:].opt()])
            cc_ar = nc.gpsimd.collective_compute("AllReduce", OP.add, replica_groups=rg,
                                         ins=[ar_in[:].opt()], outs=[ar_out[:].opt()])
            cc_a2a = nc.gpsimd.collective_compute("AllToAll", OP.bypass, replica_groups=rg,
                                         ins=[a2a_in[:].opt()], outs=[a2a_out[:].opt()])
            add_dep_helper(cc_fs.ins, cc_at.ins, reason="cc order")
            add_dep_helper(cc_ar.ins, cc_fs.ins, reason="cc order")
            add_dep_helper(cc_a2a.ins, cc_ar.ins, reason="cc order")

            # ---- fitness per edge (union gather from Fraw) ----
            for rt in range(RT):
                r0 = rt * P
                ut = sp.tile([P, K_csr], U16, tag=f"ut{rt}", name=f"ut{rt}")
                nc.sync.dma_start(ut[:], uni_in[r0:r0 + P, :])
                g = bcs.tile([P, 16 * K_csr], F32, tag="gath")
                nc.gpsimd.indirect_copy(g[:], fraw[rt][:], ut[:], True)
                gv = g[:].rearrange("p (b s) -> p s b", b=16)
                mv = m16[:].unsqueeze(1).to_broadcast([P, K_csr, 16])
                g2 = bcs.tile([P, 16 * K_csr], F32, tag="gath2")
                g2v = g2[:].rearrange("p (b s) -> p s b", b=16)
                nc.vector.tensor_tensor(out=g2v, in0=gv, in1=mv, op=OP.mult)
                ft = sp.tile([P, K_csr], F32, tag=f"ft{rt}", name=f"ft{rt}")
                nc.vector.tensor_reduce(out=ft[:], in_=g2v, axis=AX.X, op=OP.add)
                nc.sync.dma_start(fit_out[r0:r0 + P, :], ft[:])



            # ---- scores (identical on every core) ----
            num_row = sp.tile([1, N], F32, tag="num_row")
            cnt_row = sp.tile([1, N], F32, tag="cnt_row")
            nc.gpsimd.dma_start(num_row[:], ar_out[0, :])
            nc.gpsimd.dma_start(cnt_row[:], ar_out[1, :])
            sc_row = sp.tile([1, N], F32, tag="sc_row")
            nc.vector.tensor_scalar_max(sc_row[:], cnt_row[:], 1.0)
            nc.vector.reciprocal(sc_row[:], sc_row[:])
            nc.vector.tensor_tensor(out=sc_row[:], in0=sc_row[:], in1=num_row[:], op=OP.mult)
            nc.gpsimd.dma_start(scr_sc[:], sc_row[:])
            scb = bcs.tile([P, N], F32, tag="bcast", bufs=1)
            nc.gpsimd.dma_start(scb[:], bass.AP(scr_sc.tensor, 0, [[0, P], [1, N]]))

            # ---- cluster mask ----
            dcj_tile = sp.tile([P, RT], F32, tag="dcjs")
            nc.sync.dma_start(dcj_tile[:], dcj_s[:, :])
            for rt in range(RT):
                st = stat[rt]
                dms = dmsp.tile([P, N], F32, tag="dms", name=f"dmsa{rt}")
                nc.vector.tensor_scalar(dms[:], jidx[:], dcj_tile[:, rt:rt + 1],
                                        None, op0=OP.is_equal)
                scr = bcs.tile([P, N], F32, tag="scr")
                # scores_own via diag extraction; m_s via masked rowmax
                nc.vector.scalar_tensor_tensor(
                    out=scr[:], in0=scb[:], scalar=1.0, in1=dms[:],
                    op0=OP.mult, op1=OP.mult, accum_out=st[:, 12:13])
                scr2 = bcs.tile([P, N], F32, tag="scr")
                nc.vector.tensor_tensor(out=scr2[:], in0=scb[:], in1=supp[rt][:], op=OP.mult)
                nc.vector.tensor_reduce(out=st[:, 0:1], in_=scr2[:], axis=AX.X, op=OP.max)
                nc.vector.tensor_reduce(out=st[:, 1:2], in_=supp[rt][:], axis=AX.X, op=OP.max)
                nc.vector.tensor_tensor(out=st[:, 2:3], in0=st[:, 12:13],
                                        in1=st[:, 0:1], op=OP.is_ge)
                nc.vector.tensor_tensor(out=st[:, 3:4], in0=st[:, 2:3],
                                        in1=st[:, 1:2], op=OP.mult)   # cm
                nc.sync.dma_start(bass.AP(agcm_in.tensor, rt * P, [[1, P]]), st[:, 3:4])
                nc.sync.dma_start(bass.AP(scr_cm.tensor, rt * P, [[1, P]]), st[:, 3:4])

            cc_cm = nc.gpsimd.collective_compute("AllGather", OP.bypass, replica_groups=rg,
                                         ins=[agcm_in[:].opt()], outs=[agcm_out[:].opt()])
            add_dep_helper(cc_cm.ins, cc_a2a.ins, reason="cc order")
            cmb = bcs.tile([P, N], F32, tag="bcast", bufs=1)
            nc.gpsimd.dma_start(cmb[:], bass.AP(agcm_out.tensor, 0, [[0, P], [1, N]]))

            # ---- in_node / diagv / col_mask / S rows ----
            nc.vector.memset(cmk_all[:], 0.0)
            for rt in range(RT):
                st = stat[rt]
                scr = bcs.tile([P, N], F32, tag="scr")
                nc.vector.tensor_tensor(out=scr[:], in0=cmb[:], in1=supp[rt][:], op=OP.mult)
                nc.vector.tensor_reduce(out=st[:, 4:5], in_=scr[:], axis=AX.X, op=OP.max)
                nc.vector.tensor_scalar(st[:, 5:6], st[:, 13:14], 0.0, None, op0=OP.is_gt)  # has_in
                nc.vector.tensor_tensor(out=st[:, 6:7], in0=st[:, 3:4], in1=st[:, 5:6], op=OP.mult)
                nc.vector.tensor_tensor(out=st[:, 7:8], in0=st[:, 4:5], in1=st[:, 6:7], op=OP.max)
                nc.vector.tensor_scalar(st[:, 8:9], st[:, 7:8], 0.0, None, op0=OP.is_gt)
                nc.vector.tensor_scalar(st[:, 9:10], st[:, 8:9], -1.0, 1.0,
                                        op0=OP.mult, op1=OP.add)          # non_in
                nc.vector.tensor_tensor(out=st[:, 10:11], in0=st[:, 3:4],
                                        in1=st[:, 9:10], op=OP.add)       # diagv
                nc.vector.tensor_tensor(out=st[:, 11:12], in0=st[:, 3:4],
                                        in1=st[:, 9:10], op=OP.max)       # col_mask
                nc.sync.dma_start(cmask_out[:, rt:rt + 1], st[:, 11:12])
                nc.sync.dma_start(bass.AP(agdv_in.tensor, rt * P, [[1, P]]), st[:, 10:11])
                nc.sync.dma_start(bass.AP(scr_cm.tensor, ROWS + rt * P, [[1, P]]), st[:, 10:11])
                for i in range(KT // RT):
                    nc.vector.tensor_copy(cmk_all[:, rt + i * RT:rt + i * RT + 1],
                                          st[:, 11:12])
                # S rows f32 (output)
                dms2 = dmsp.tile([P, N], F32, tag="dms", name=f"dmsb{rt}")
                nc.vector.tensor_scalar(dms2[:], jidx[:], dcj_tile[:, rt:rt + 1],
                                        None, op0=OP.is_equal)
                nc.vector.tensor_tensor(out=fs[rt][:], in0=fs[rt][:], in1=cmb[:], op=OP.mult)
                nc.vector.scalar_tensor_tensor(
                    out=fs[rt][:], in0=dms2[:], scalar=st[:, 10:11], in1=fs[rt][:],
                    op0=OP.mult, op1=OP.add)
                nc.sync.dma_start(s_out[rt * P:(rt + 1) * P, :], fs[rt][:])

            dmsp_cm.__exit__(None, None, None)
            bcs_cm.__exit__(None, None, None)
            bc2_cm.__exit__(None, None, None)
            bc1_cm.__exit__(None, None, None)

            # ================= phase D =================
            with tc.tile_pool(name="dp", bufs=1) as dp, \
                 tc.tile_pool(name="dps", bufs=4) as dps:
                cc_dv = nc.gpsimd.collective_compute("AllGather", OP.bypass, replica_groups=rg,
                                             ins=[agdv_in[:].opt()], outs=[agdv_out[:].opt()])
                add_dep_helper(cc_dv.ins, cc_cm.ins, reason="cc order")
                fscol = [dp.tile([P, ROWS], BF16, tag=f"fscol{k}", name=f"fscol{k}") for k in range(KT)]
                asb = [dp.tile([P, ROWS], BF16, tag=f"asb{k}", name=f"asb{k}") for k in range(KT)]
                fs_v = bass.AP(a2a_out.tensor, 0, [[ROWS, N], [1, ROWS]])
                for k in range(KT):
                    nc.sync.dma_start(fscol[k][:], fs_v[k * P:(k + 1) * P, :])

                dcje_t = sp.tile([P, KT], F32, tag="dcje")
                nc.sync.dma_start(dcje_t[:], dcj_e[:, :])
                cm_ownb = dp.tile([P, ROWS], F32, tag="cm_ownb")
                dv_ownb = dp.tile([P, ROWS], F32, tag="dv_ownb")
                cm_le = sp.tile([P, KT], F32, tag="cm_le")
                dv_le = sp.tile([P, KT], F32, tag="dv_le")

                # ---- mm1: Y1 = A @ FS[:, own] ----
                with tc.tile_pool(name="mm1_ps", bufs=1, space="PSUM") as mmps:
                    ps1 = [mmps.tile([P, ROWS], F32, tag=f"mm1_{i}", name=f"mm1_{i}", space="PSUM")
                           for i in range(8)]
                    for half in range(2):
                        for k in range(KT):
                            atrow = dps.tile([P, N], BF16, tag="atrow")
                            nc.sync.dma_start(atrow[:], agat_out[k * P:(k + 1) * P, :])
                            for i8 in range(8):
                                it = half * 8 + i8
                                nc.tensor.matmul(
                                    ps1[i8][:], atrow[:, it * P:(it + 1) * P], fscol[k][:],
                                    start=(k == 0), stop=(k == KT - 1))
                        if half == 0:
                            # issue late small loads on gpsimd queue (not sync) so
                            # they cannot stall the matmul input stream
                            nc.gpsimd.dma_start(cm_ownb[:], bass.AP(scr_cm.tensor, 0, [[0, P], [1, ROWS]]))
                            nc.gpsimd.dma_start(dv_ownb[:], bass.AP(scr_cm.tensor, ROWS, [[0, P], [1, ROWS]]))
                            nc.gpsimd.dma_start(cm_le[:], bass.AP(agcm_out.tensor, 0, [[1, P], [P, KT]]))
                            nc.gpsimd.dma_start(dv_le[:], bass.AP(agdv_out.tensor, 0, [[1, P], [P, KT]]))
                        for i8 in range(8):
                            it = half * 8 + i8
                            # AS = cm_own*Y1 + A[:,own]*diagv_own
                            t1 = dps.tile([P, ROWS], F32, tag="t1", name=f"t1_{it}")
                            nc.vector.tensor_tensor(out=t1[:], in0=acol[it][:],
                                                    in1=dv_ownb[:], op=OP.mult)
                            t2 = dps.tile([P, ROWS], F32, tag="t2", name=f"t2_{it}")
                            nc.vector.tensor_tensor(out=t2[:], in0=ps1[i8][:],
                                                    in1=cm_ownb[:], op=OP.mult)
                            nc.vector.tensor_tensor(out=asb[it][:], in0=t2[:],
                                                    in1=t1[:], op=OP.add)

                # ---- mm2: Emat = cm[i]*(FS^T @ AS) + diagv[i]*AS ----
                with tc.tile_pool(name="mm2_ps", bufs=1, space="PSUM") as mmps2:
                    ps2 = [mmps2.tile([P, ROWS], F32, tag=f"mm2_{i}", name=f"mm2_{i}", space="PSUM")
                           for i in range(8)]
                    for half in range(2):
                        for k in range(KT):
                            srow = dps.tile([P, N], BF16, tag="srow")
                            nc.sync.dma_start(srow[:], agfs_out[k * P:(k + 1) * P, :])
                            for i8 in range(8):
                                it = half * 8 + i8
                                nc.tensor.matmul(
                                    ps2[i8][:], srow[:, it * P:(it + 1) * P], asb[k][:],
                                    start=(k == 0), stop=(k == KT - 1))
                        for i8 in range(8):
                            it = half * 8 + i8
                            dme = dps.tile([P, ROWS], F32, tag="dme", name=f"dme{it}")
                            nc.vector.tensor_scalar(dme[:], jidx[:, 0:ROWS],
                                                    dcje_t[:, it:it + 1], None, op0=OP.is_equal)
                            t1 = dps.tile([P, ROWS], F32, tag="t1", name=f"t1e_{it}")
                            nc.vector.tensor_scalar(t1[:], asb[it][:], dv_le[:, it:it + 1],
                                                    None, op0=OP.mult)
                            eo = dps.tile([P, ROWS], F32, tag="eo", name=f"eo{it}")
                            nc.vector.scalar_tensor_tensor(
                                out=eo[:], in0=ps2[i8][:], scalar=cm_le[:, it:it + 1],
                                in1=t1[:], op0=OP.mult, op1=OP.add)
                            de = sp.tile([P, 4], F32, tag=f"de{it % 4}", name=f"de{it}")
                            scr_e = dps.tile([P, ROWS], F32, tag="scr_e", name=f"scr_e{it}")
                            nc.vector.scalar_tensor_tensor(
                                out=scr_e[:], in0=eo[:], scalar=1.0, in1=dme[:],
                                op0=OP.mult, op1=OP.mult, accum_out=de[:, 0:1])
                            nc.vector.tensor_scalar(de[:, 1:2], de[:, 0:1], 0.0, None,
                                                    op0=OP.is_equal)
                            nc.vector.tensor_tensor(out=de[:, 2:3], in0=de[:, 1:2],
                                                    in1=cmk_all[:, it:it + 1], op=OP.mult)
                            nc.vector.scalar_tensor_tensor(
                                out=eo[:], in0=dme[:], scalar=de[:, 2:3], in1=eo[:],
                                op0=OP.mult, op1=OP.add)
                            nc.sync.dma_start(emat_out[it * P:(it + 1) * P, :], eo[:])

                # ---- pooled ----
                with tc.tile_pool(name="mm3_ps", bufs=2, space="PSUM") as mmps3:
                    mcol = [dp.tile([P, ROWS], BF16, tag=f"mcol{k}", name=f"mcol{k}") for k in range(KT)]
                    for k in range(KT):
                        dme = dps.tile([P, ROWS], F32, tag="dme", name=f"dmep{k}")
                        nc.vector.tensor_scalar(dme[:], jidx[:, 0:ROWS],
                                                dcje_t[:, k:k + 1], None, op0=OP.is_equal)
                        sc_t = dps.tile([P, ROWS], F32, tag="sc_t", name=f"sc_t{k}")
                        nc.vector.tensor_tensor(out=sc_t[:], in0=fscol[k][:],
                                                in1=cm_ownb[:], op=OP.mult)
                        nc.vector.scalar_tensor_tensor(
                            out=sc_t[:], in0=dme[:], scalar=dv_le[:, k:k + 1], in1=sc_t[:],
                            op0=OP.mult, op1=OP.add)
                        nc.vector.tensor_scalar(mcol[k][:], sc_t[:], 0.0, None, op0=OP.is_gt)
                    for jt in range(RT):
                        pl_ps = mmps3.tile([P, D], F32, tag="pool_ps", space="PSUM")
                        pc_ps = mmps3.tile([P, 2], F32, tag="cnt_ps", space="PSUM")
                        for k in range(KT):
                            nc.tensor.matmul(pl_ps[:], mcol[k][:, jt * P:(jt + 1) * P],
                                             embb[k][:], start=(k == 0), stop=(k == KT - 1))
                            nc.tensor.matmul(pc_ps[:, 0:1], mcol[k][:, jt * P:(jt + 1) * P],
                                             ones_b[:], start=(k == 0), stop=(k == KT - 1))
                        cn = sp.tile([P, 4], F32, tag=f"cn{jt}", name=f"cn{jt}")
                        nc.vector.tensor_scalar_max(cn[:, 0:1], pc_ps[:, 0:1], 1.0)
                        nc.vector.reciprocal(cn[:, 1:2], cn[:, 0:1])
                        nc.vector.tensor_tensor(out=cn[:, 2:3], in0=cn[:, 1:2],
                                                in1=stat[jt][:, 11:12], op=OP.mult)
                        po = dps.tile([P, D], F32, tag="po")
                        nc.scalar.activation(po[:], pl_ps[:], ACT.Copy, scale=cn[:, 2:3])
                        nc.sync.dma_start(pooled_out[jt * P:(jt + 1) * P, :], po[:])

    nc.compile()
    return nc


# --------------------------------------------------------------------------
# host prep / assembly
# --------------------------------------------------------------------------
def _prep(edge_index):
    src = np.asarray(edge_index[0], np.int64)
    dst = np.asarray(edge_index[1], np.int64)
    key = src * N + dst
    uniq, inv, mult = np.unique(key, return_inverse=True, return_counts=True)
    usrc = (uniq // N).astype(np.int64)
    udst = (uniq % N).astype(np.int64)
    Eu = len(uniq)

    deg = np.bincount(usrc, minlength=N)
    K_csr = max(2, int(np.ceil(deg.max() / 2) * 2))
    row_start = np.zeros(N + 1, np.int64)
    np.cumsum(deg, out=row_start[1:])
    pos = np.arange(Eu) - row_start[usrc]

    dstM = np.full((N, K_csr), -1, np.int64)
    dstM[usrc, pos] = udst
    multM = np.zeros((N, K_csr), np.float32)
    multM[usrc, pos] = mult

    h0 = np.where((dstM >= 0) & (dstM < N // 2), dstM, -1).astype(np.int16)
    h1 = np.where(dstM >= N // 2, dstM - N // 2, -1).astype(np.int16)

    dst_clip = np.maximum(dstM, 0).astype(np.uint16)
    uni = np.zeros((N, K_csr), np.uint16)
    wrap = (np.arange(K_csr)[None, :] * 16 + np.arange(16)[:, None]).reshape(-1)
    for g in range(N // 16):
        ul = dst_clip[g * 16:(g + 1) * 16].reshape(-1)
        uni[g * 16:(g + 1) * 16] = ul[wrap].reshape(16, K_csr)

    order2 = np.lexsort((usrc, udst))
    csrc = usrc[order2]
    cdst = udst[order2]
    cmult = mult[order2]
    ideg = np.bincount(cdst, minlength=N)
    K_csc = max(2, int(np.ceil(ideg.max() / 2) * 2))
    cstart = np.zeros(N + 1, np.int64)
    np.cumsum(ideg, out=cstart[1:])
    cpos = np.arange(Eu) - cstart[cdst]
    srcM = np.full((N, K_csc), -1, np.int64)
    srcM[cdst, cpos] = csrc
    cmultM = np.zeros((N, K_csc), np.float32)
    cmultM[cdst, cpos] = cmult
    c0 = np.where((srcM >= 0) & (srcM < N // 2), srcM, -1).astype(np.int16)
    c1 = np.where(srcM >= N // 2, srcM - N // 2, -1).astype(np.int16)

    in_maps = []
    rows_p = np.arange(P)
    for c in range(NCORES):
        base = c * ROWS
        sl = slice(base, base + ROWS)
        dcj_sv = (base + P * np.arange(RT)[None, :] + rows_p[:, None]).astype(np.float32)
        dcj_ev = (P * np.arange(KT)[None, :] + rows_p[:, None] - base).astype(np.float32)
        in_maps.append({
            "csr_h0": h0[sl].copy(), "csr_h1": h1[sl].copy(),
            "csr_mult": multM[sl].astype(ml_dtypes.bfloat16),
            "uni_idx": uni[sl].copy(),
            "csc_h0": c0[sl].copy(), "csc_h1": c1[sl].copy(),
            "csc_mult": cmultM[sl].astype(ml_dtypes.bfloat16),
            "diag_cj_s": np.ascontiguousarray(dcj_sv),
            "diag_cj_e": np.ascontiguousarray(dcj_ev),
        })
    fit_row = usrc[inv]
    fit_slot = pos[inv]
    return in_maps, K_csr, K_csc, fit_row, fit_slot


def prepare(embedding, edge_index):
    emb = np.ascontiguousarray(np.asarray(embedding, np.float32))
    ei = np.asarray(edge_index)
    in_maps, K_csr, K_csc, fit_row, fit_slot = _prep(ei)
    for c in range(NCORES):
        in_maps[c]["emb"] = emb
        in_maps[c]["emb_own"] = np.ascontiguousarray(emb[c * ROWS:(c + 1) * ROWS])
    ck = (K_csr, K_csc)
    if ck not in _cache:
        _cache[ck] = _build(K_csr, K_csc)
    return _cache[ck], in_maps, fit_row, fit_slot


def assemble(results, fit_row, fit_slot):
    r = results
    S = np.concatenate([np.asarray(r[c]["s_out"]) for c in range(NCORES)], axis=0)
    Emat = np.concatenate([np.asarray(r[c]["emat_out"]) for c in range(NCORES)], axis=1)
    pooled = np.concatenate([np.asarray(r[c]["pooled_out"]) for c in range(NCORES)], axis=0)
    cmask = np.concatenate(
        [np.asarray(r[c]["cmask_out"]).T.reshape(-1) for c in range(NCORES)]) > 0.5
    fit_all = np.concatenate([np.asarray(r[c]["fit_out"]) for c in range(NCORES)], axis=0)
    fitness = np.ascontiguousarray(fit_all[fit_row, fit_slot].astype(np.float32))
    return (np.ascontiguousarray(pooled.astype(np.float32)),
            np.ascontiguousarray(Emat.astype(np.float32)),
            np.ascontiguousarray(S.astype(np.float32)),
            fitness, cmask)


def kernel(embedding, edge_index):
    nc, in_maps, fit_row, fit_slot = prepare(embedding, edge_index)
    res = run_bass_kernel_spmd(nc, in_maps, core_ids=list(range(NCORES)))
    return assemble(res.results, fit_row, fit_slot)


# revision 25
# speedup vs baseline: 1.0775x; 1.0071x over previous
"""Adaptive graph pooling (gnn_message_passing) on 8 TRN2 NeuronCores.

Sharding: nodes 256-per-core.  Host work is limited to sharding / index prep
(CSR/CSC bucketing of the edge list) and output assembly.

v2 pipeline (collectives start as early as their inputs allow; S-dependent
matmuls are decomposed so the big AllGathers ship cm-independent tensors):

  S = FS .* cm[col] + diag(diagv)          (FS = multiplicity-weighted fitness)
  AS[:,own]   = cm[own]*(A @ FS[:,own]) + A[:,own]*diagv[own]
  Emat[:,own] = cm[row] * (FS^T @ AS) + diagv[row] * AS

  collective order: AG(A^T) | AllReduce(colsums) | A2A(FS,A cols) | AG(cm)
                    | AG(FS) | AG(diagv)
  mm1 = A @ FS[:,own] runs while AG(FS)/AG(cm) are on the wire.
"""
import sys
if '/opt/trn_rl_repo' not in sys.path:
    sys.path.insert(0, '/opt/trn_rl_repo')

import numpy as np
import ml_dtypes

import concourse.bass as bass
import concourse.tile as tile
from concourse.tile import add_dep_helper
from concourse import bacc, mybir, library_config
from concourse.bass_utils import run_bass_kernel_spmd

F32 = mybir.dt.float32
BF16 = mybir.dt.bfloat16
I16 = mybir.dt.int16
U16 = mybir.dt.uint16
AX = mybir.AxisListType
OP = mybir.AluOpType
ACT = mybir.ActivationFunctionType

N = 2048
D = 512
NCORES = 8
ROWS = N // NCORES
P = 128
RT = ROWS // P
KT = N // P
DT = D // P

_cache = {}
_SKIP_LIB = False


def _rsqrt(nc, ss, col):
    """ss[:, col] = sumsq -> returns AP of 1/max(sqrt(ss),1e-12)."""
    c = col
    nc.scalar.activation(ss[:, c + 1:c + 2], ss[:, c:c + 1], ACT.Sqrt)
    nc.vector.tensor_scalar_max(ss[:, c + 2:c + 3], ss[:, c + 1:c + 2], 1e-12)
    nc.vector.reciprocal(ss[:, c + 3:c + 4], ss[:, c + 2:c + 3])
    return ss[:, c + 3:c + 4]


def _build(K_csr, K_csc):
    import kernel_build
    return kernel_build.build(K_csr, K_csc)


# --------------------------------------------------------------------------
# host prep / assembly
# --------------------------------------------------------------------------
def _prep(edge_index):
    src = np.asarray(edge_index[0], np.int64)
    dst = np.asarray(edge_index[1], np.int64)
    key = src * N + dst
    uniq, inv, mult = np.unique(key, return_inverse=True, return_counts=True)
    usrc = (uniq // N).astype(np.int64)
    udst = (uniq % N).astype(np.int64)
    Eu = len(uniq)

    deg = np.bincount(usrc, minlength=N)
    K_csr = max(2, int(np.ceil(deg.max() / 2) * 2))
    row_start = np.zeros(N + 1, np.int64)
    np.cumsum(deg, out=row_start[1:])
    pos = np.arange(Eu) - row_start[usrc]

    dstM = np.full((N, K_csr), -1, np.int64)
    dstM[usrc, pos] = udst
    multM = np.zeros((N, K_csr), np.float32)
    multM[usrc, pos] = mult

    h0 = np.where((dstM >= 0) & (dstM < N // 2), dstM, -1).astype(np.int16)
    h1 = np.where(dstM >= N // 2, dstM - N // 2, -1).astype(np.int16)

    dst_clip = np.maximum(dstM, 0).astype(np.uint16)
    uni = np.zeros((N, K_csr), np.uint16)
    wrap = (np.arange(K_csr)[None, :] * 16 + np.arange(16)[:, None]).reshape(-1)
    for g in range(N // 16):
        ul = dst_clip[g * 16:(g + 1) * 16].reshape(-1)
        uni[g * 16:(g + 1) * 16] = ul[wrap].reshape(16, K_csr)

    order2 = np.lexsort((usrc, udst))
    csrc = usrc[order2]
    cdst = udst[order2]
    cmult = mult[order2]
    ideg = np.bincount(cdst, minlength=N)
    K_csc = max(2, int(np.ceil(ideg.max() / 2) * 2))
    cstart = np.zeros(N + 1, np.int64)
    np.cumsum(ideg, out=cstart[1:])
    cpos = np.arange(Eu) - cstart[cdst]
    srcM = np.full((N, K_csc), -1, np.int64)
    srcM[cdst, cpos] = csrc
    cmultM = np.zeros((N, K_csc), np.float32)
    cmultM[cdst, cpos] = cmult
    c0 = np.where((srcM >= 0) & (srcM < N // 2), srcM, -1).astype(np.int16)
    c1 = np.where(srcM >= N // 2, srcM - N // 2, -1).astype(np.int16)

    in_maps = []
    rows_p = np.arange(P)
    for c in range(NCORES):
        base = c * ROWS
        sl = slice(base, base + ROWS)
        dcj_sv = (base + P * np.arange(RT)[None, :] + rows_p[:, None]).astype(np.float32)
        dcj_ev = (P * np.arange(KT)[None, :] + rows_p[:, None] - base).astype(np.float32)
        in_maps.append({
            "csr_h0": h0[sl].copy(), "csr_h1": h1[sl].copy(),
            "csr_mult": multM[sl].astype(ml_dtypes.bfloat16),
            "uni_idx": uni[sl].copy(),
            "csc_h0": c0[sl].copy(), "csc_h1": c1[sl].copy(),
            "csc_mult": cmultM[sl].astype(ml_dtypes.bfloat16),
            "diag_cj_s": np.ascontiguousarray(dcj_sv),
            "diag_cj_e": np.ascontiguousarray(dcj_ev),
        })
    fit_row = usrc[inv]
    fit_slot = pos[inv]
    return in_maps, K_csr, K_csc, fit_row, fit_slot


def prepare(embedding, edge_index):
    emb = np.ascontiguousarray(np.asarray(embedding, np.float32))
    ei = np.asarray(edge_index)
    in_maps, K_csr, K_csc, fit_row, fit_slot = _prep(ei)
    for c in range(NCORES):
        in_maps[c]["emb"] = emb
        in_maps[c]["emb_own"] = np.ascontiguousarray(emb[c * ROWS:(c + 1) * ROWS])
    ck = (K_csr, K_csc)
    if ck not in _cache:
        _cache[ck] = _build(K_csr, K_csc)
    return _cache[ck], in_maps, fit_row, fit_slot


def assemble(results, fit_row, fit_slot):
    r = results
    S = np.concatenate([np.asarray(r[c]["s_out"]) for c in range(NCORES)], axis=0)
    Emat = np.concatenate([np.asarray(r[c]["emat_out"]) for c in range(NCORES)], axis=1)
    pooled = np.concatenate([np.asarray(r[c]["pooled_out"]) for c in range(NCORES)], axis=0)
    cmask = np.concatenate(
        [np.asarray(r[c]["cmask_out"]).T.reshape(-1) for c in range(NCORES)]) > 0.5
    fit_all = np.concatenate([np.asarray(r[c]["fit_out"]) for c in range(NCORES)], axis=0)
    fitness = np.ascontiguousarray(fit_all[fit_row, fit_slot].astype(np.float32))
    return (np.ascontiguousarray(pooled.astype(np.float32)),
            np.ascontiguousarray(Emat.astype(np.float32)),
            np.ascontiguousarray(S.astype(np.float32)),
            fitness, cmask)


def kernel(embedding, edge_index):
    nc, in_maps, fit_row, fit_slot = prepare(embedding, edge_index)
    res = run_bass_kernel_spmd(nc, in_maps, core_ids=list(range(NCORES)))
    return assemble(res.results, fit_row, fit_slot)
